# revision 22
# baseline (speedup 1.0000x reference)
"""CaptioningRNN (LSTM + tiny spatial attention) Trainium2 kernel, v2.

Contract: kernel(**inputs) takes FULL inputs (numpy), returns FULL output
(N, T, H) float32.  Internally: data-parallel over batch N across 8
NeuronCores (16 sequences per core, zero cross-core traffic).

Per-core algorithm:
  phase 0a: P[t] = x_t @ Wx (+ b) for all t (PE, bf16, gate-permuted
            columns) -> DRAM
  phase 0b: B[n,m,:] = A[n,:,m] @ Wattn (PE, bf16) -> SBUF resident.
            This turns the per-step attn@Wattn matmul into a rank-16
            contraction w.B with a block-diagonal stationary.
  phase 1: 512 sequential LSTM steps, all engines pipelined:
    - gates PSUM bank jt holds [i|f|o|g] for H-slice jt (columns
      permuted host-side) so each bank postprocesses independently
    - softmax without Exp: e^s = (1+tanh(s/2))/(1-tanh(s/2)) keeps
      every Activation op in the sigmoid/tanh table set (no
      ACT_TABLE_LOAD thrash)
    - PSUM gate buffers alternate partitions 0:16 / 32:48 by step
      parity to avoid write-after-read stalls
"""

import sys
import numpy as np

sys.path.insert(0, "/opt/trn_rl_repo")

import ml_dtypes

BF16 = ml_dtypes.bfloat16

N, T, D, H, M = 128, 512, 512, 512, 16
NCORES = 8
NL = N // NCORES          # 16 sequences per core
KC = 4                    # 512 = 4 chunks of 128 (contraction dims)
J = 4 * H                 # 2048 gate columns
TB = 8                    # time steps per phase-0 row block
RB = NL * T // 128        # phase-0 row blocks (rows = tt*NL + n)
NG = 2                    # sequence groups of 8 for the B contraction

_CACHE = {}


def build(t_steps=T, has_bias=False):
    from concourse import bacc, mybir
    import concourse.tile as tile

    f32 = mybir.dt.float32
    bf16 = mybir.dt.bfloat16
    mult = mybir.AluOpType.mult
    add = mybir.AluOpType.add
    AF = mybir.ActivationFunctionType
    AX = mybir.AxisListType.X

    rb = NL * t_steps // 128

    nc = bacc.Bacc("TRN2", target_bir_lowering=False, debug=False,
                   num_devices=NCORES)

    # ---- I/O -----------------------------------------------------------
    xs = nc.dram_tensor("xs", [rb, 128, KC, 128], bf16, kind="ExternalInput")
    at_d = nc.dram_tensor("at", [128, KC, NL, M], bf16, kind="ExternalInput")
    wx_d = nc.dram_tensor("wx", [128, KC, J], bf16, kind="ExternalInput")
    wh_d = nc.dram_tensor("wh", [128, KC, J], bf16, kind="ExternalInput")
    wa_d = nc.dram_tensor("wa", [128, KC, J], bf16, kind="ExternalInput")
    h0_d = nc.dram_tensor("h0t", [128, KC, NL], bf16, kind="ExternalInput")
    c0_d = nc.dram_tensor("c0", [NL, H], f32, kind="ExternalInput")
    id_d = nc.dram_tensor("ident", [NL, NL], bf16, kind="ExternalInput")
    oc_d = nc.dram_tensor("ones_col", [128, 1], bf16, kind="ExternalInput")
    m0_d = nc.dram_tensor("m0", [128, NG, NL], bf16, kind="ExternalInput")
    if has_bias:
        b_d = nc.dram_tensor("bvec", [1, J], f32, kind="ExternalInput")
    p_d = nc.dram_tensor("pbuf", [rb, 128, J], bf16)
    out_d = nc.dram_tensor("out", [NL, t_steps, H], bf16,
                           kind="ExternalOutput")

    half_inv_sqrt_h = float(0.5 / np.sqrt(H))

    from contextlib import ExitStack
    with tile.TileContext(nc) as tc, ExitStack() as stack:
        # ---- persistent constants -------------------------------------
        cpool = stack.enter_context(tc.tile_pool(name="consts", bufs=1))
        wh_s = cpool.tile([128, KC, J], bf16)
        at_s = cpool.tile([128, KC, NL, M], bf16)
        b_s = cpool.tile([128, NG, J], bf16)          # B (attention basis)
        h0_s = cpool.tile([128, KC, NL], bf16)
        id_s = cpool.tile([NL, NL], bf16)
        oc_s = cpool.tile([128, 1], bf16)
        m0_s = cpool.tile([128, NG, NL], bf16)
        nc.sync.dma_start(out=wh_s[:, :, :], in_=wh_d.ap()[:, :, :])
        nc.sync.dma_start(out=at_s[:, :, :, :], in_=at_d.ap()[:, :, :, :])
        nc.sync.dma_start(out=h0_s[:, :, :], in_=h0_d.ap()[:, :, :])
        nc.sync.dma_start(out=id_s[:, :], in_=id_d.ap()[:, :])
        nc.sync.dma_start(out=oc_s[:, :], in_=oc_d.ap()[:, :])
        nc.sync.dma_start(out=m0_s[:, :, :], in_=m0_d.ap()[:, :, :])

        # ---- phase 0a: P = x @ Wx (+ b) -------------------------------
        with tc.tile_pool(name="ph0", bufs=1) as p0c, \
             tc.tile_pool(name="ph0x", bufs=3) as p0x, \
             tc.tile_pool(name="ph0o", bufs=3) as p0o, \
             tc.tile_pool(name="ps0", bufs=2, space="PSUM") as ps0:
            wx_s = p0c.tile([128, KC, J], bf16)
            wa_s = p0c.tile([128, KC, J], bf16)
            nc.sync.dma_start(out=wx_s[:, :, :], in_=wx_d.ap()[:, :, :])
            nc.sync.dma_start(out=wa_s[:, :, :], in_=wa_d.ap()[:, :, :])
            if has_bias:
                bf_s = p0c.tile([1, J], f32)
                nc.sync.dma_start(out=bf_s[:, :], in_=b_d.ap()[:, :])
                bb_s = p0c.tile([1, J], bf16)
                nc.vector.tensor_copy(bb_s[:, :], bf_s[:, :])
                or_s = p0c.tile([1, 128], bf16)
                nc.vector.memset(or_s[:, :], 1.0)
                psb = ps0.tile([128, J], f32, tag="psb")
                for jt in range(4):
                    nc.tensor.matmul(psb[:, jt * 512:(jt + 1) * 512],
                                     or_s[:, :],
                                     bb_s[:, jt * 512:(jt + 1) * 512],
                                     start=True, stop=True)
                brep = p0c.tile([128, J], bf16)
                nc.vector.tensor_copy(brep[:, :], psb[:, :])

            for b_i in range(rb):
                xt = p0x.tile([128, KC, 128], bf16, tag="xt")
                nc.sync.dma_start(out=xt[:, :, :], in_=xs.ap()[b_i, :, :, :])
                psp = ps0.tile([128, J], f32, tag="psp")
                for kc in range(KC):
                    for jt in range(4):
                        nc.tensor.matmul(
                            psp[:, jt * 512:(jt + 1) * 512],
                            xt[:, kc, :],
                            wx_s[:, kc, jt * 512:(jt + 1) * 512],
                            start=(kc == 0), stop=(kc == KC - 1))
                pout = p0o.tile([128, J], bf16, tag="pout")
                for jt in range(4):
                    sl = slice(jt * 512, (jt + 1) * 512)
                    if has_bias:
                        nc.vector.tensor_tensor(pout[:, sl], psp[:, sl],
                                                brep[:, sl], add)
                    elif jt in (1, 3):
                        nc.scalar.copy(pout[:, sl], psp[:, sl])
                    else:
                        nc.vector.tensor_copy(pout[:, sl], psp[:, sl])
                nc.sync.dma_start(out=p_d.ap()[b_i, :, :], in_=pout[:, :])

            # ---- phase 0b: B[(i,m), g, :] = sum_h A[g8+i, h, m] Wattn[h, :]
            for g in range(NG):
                for jt in range(4):
                    psB_f = ps0.tile([128, J], f32, tag="psp", name="psB_f")
                    psB = psB_f[:, 0:512]
                    for kc in range(KC):
                        nc.tensor.matmul(
                            psB[:, :],
                            at_s[:, kc, g * 8:(g + 1) * 8, :],
                            wa_s[:, kc, jt * 512:(jt + 1) * 512],
                            start=(kc == 0), stop=(kc == KC - 1))
                    sl = slice(jt * 512, (jt + 1) * 512)
                    if jt in (1, 3):
                        nc.scalar.copy(b_s[:, g, sl], psB[:, :])
                    else:
                        nc.vector.tensor_copy(b_s[:, g, sl], psB[:, :])

        # persistent PSUM (allocated after phase 0 frees its banks)
        pp = stack.enter_context(tc.tile_pool(name="ppsum", bufs=1,
                                              space="PSUM"))
        # separate tiles per gate bank / transpose chunk: byte-range dep
        # tracking linearizes multi-partition slices, so slices of one big
        # tile would false-conflict across banks and serialize the pipeline
        # psa rotates over 4 partition slots (0/32/64/96) so a step's gate
        # writes only WAR against step t-4's activation reads
        psa_b = [pp.tile([80, 512], f32, name=f"psa{i}") for i in range(4)]
        psz_r = [pp.tile([1, NL, M], f32, name=f"psz{i}") for i in range(2)]
        pst_p = [pp.tile([128, 2, NL + 2], bf16, name=f"pstp{i}")
                 for i in range(2)]
        pst_b = [pst_p[0][:, 0, 0:NL], pst_p[0][:, 1, 0:NL],
                 pst_p[1][:, 0, 0:NL], pst_p[1][:, 1, 0:NL]]
        # w-column transposes live in the spare columns of pst_p[1]
        wcol = [pst_p[1][:, 0, NL:NL + 1], pst_p[1][:, 1, NL:NL + 1]]

        # ---- phase 1: recurrence --------------------------------------
        with tc.tile_pool(name="state", bufs=3) as stp, \
             tc.tile_pool(name="work", bufs=3) as wk, \
             tc.tile_pool(name="pin", bufs=3) as pin, \
             tc.tile_pool(name="hout", bufs=3) as hop:

            c_cur = []
            for jt in range(4):
                c_j = stp.tile([NL, 128], f32, tag=f"c{jt}", name="c_j")
                nc.sync.dma_start(out=c_j[:, :],
                                  in_=c0_d.ap()[:, jt * 128:(jt + 1) * 128])
                c_cur.append(c_j)
            hT_cur = h0_s

            # state carried across iterations of the emission loop
            h_prev = None      # h_out tile of previous step (for transposes)
            hT_next = None

            for t in range(t_steps):
                p0 = (t % 3) * 32
                psz = psz_r[t % 2]

                p_t = pin.tile([NL, J], bf16, tag="pt")
                b_i, tt = divmod(t, TB)
                nc.sync.dma_start(out=p_t[:, :],
                                  in_=p_d.ap()[b_i, tt * NL:(tt + 1) * NL, :])

                s2 = [wk.tile([128, NL, M], bf16, tag=f"s2k{kc}",
                              name="s2_k") for kc in range(KC)]
                for kc in (range(KC) if t == 0 else (0,)):
                    eng = nc.vector if kc == KC - 1 else nc.gpsimd
                    eng.tensor_tensor(
                        s2[kc][:, :, :], at_s[:, kc, :, :],
                        hT_cur[:, kc, :, None].broadcast_to([128, NL, M]),
                        mult)

                # -- Wh matmuls, interleaved with prev-step transposes and
                #    the attention colsum
                for kc in range(KC):
                    if t > 0 and kc > 0:
                        # transpose chunk kc of h_t (made by prev step)
                        nc.tensor.transpose(
                            pst_b[kc][:, :],
                            h_prev[:, kc * 128:(kc + 1) * 128], id_s[:, :])
                        nc.vector.tensor_copy(hT_cur[:, kc, :], pst_b[kc][:, :])
                        eng = nc.vector if kc == KC - 1 else nc.gpsimd
                        eng.tensor_tensor(
                            s2[kc][:, :, :], at_s[:, kc, :, :],
                            hT_cur[:, kc, :, None].broadcast_to([128, NL, M]),
                            mult)
                    if kc < KC - 1:
                        # Wh for the last chunk is held back to fill the
                        # PE idle window while the softmax chain runs
                        for jt in range(4):
                            nc.tensor.matmul(
                                psa_b[jt][p0:p0 + 16, :],
                                hT_cur[:, kc, :],
                                wh_s[:, kc, jt * 512:(jt + 1) * 512],
                                start=(kc == 0), stop=False)
                    if kc > 0:
                        # colsum for chunk kc-1 (s2 ready by now)
                        nc.tensor.matmul(psz[:, :, :], oc_s[:, :],
                                         s2[kc - 1][:, :, :],
                                         start=(kc == 1), stop=False)
                nc.tensor.matmul(psz[:, :, :], oc_s[:, :], s2[KC - 1][:, :, :],
                                 start=False, stop=True)

                # -- P inject + Wh kc3 (PE fillers while softmax runs)
                for jt in range(4):
                    nc.tensor.matmul(psa_b[jt][p0:p0 + 16, :],
                                     id_s[:, :],
                                     p_t[:, jt * 512:(jt + 1) * 512],
                                     start=False, stop=False)
                for jt in range(4):
                    nc.tensor.matmul(
                        psa_b[jt][p0:p0 + 16, :],
                        hT_cur[:, KC - 1, :],
                        wh_s[:, KC - 1, jt * 512:(jt + 1) * 512],
                        start=False, stop=False)

                # -- softmax via tanh: e^s = (1+u)/(1-u), u = tanh(s/2)
                u_t = wk.tile([1, NL, M], f32, tag="u")
                nc.scalar.activation(u_t[:, :, :], psz[:, :, :], AF.Tanh,
                                     scale=half_inv_sqrt_h)
                den = wk.tile([1, NL, M], f32, tag="den")
                nc.vector.tensor_scalar(den[:, :, :], u_t[:, :, :],
                                        -1.0, 1.0, mult, add)
                rden = wk.tile([1, NL, M], f32, tag="rden")
                nc.vector.reciprocal_approx_fast(rden[:, :, :], den[:, :, :])
                r_t = wk.tile([1, NL, M], f32, tag="r")
                nc.vector.tensor_scalar(r_t[:, :, :], rden[:, :, :],
                                        2.0, -1.0, mult, add)
                ssum = wk.tile([1, NL, 1], f32, tag="ssum")
                nc.vector.tensor_reduce(ssum[:, :, :], r_t[:, :, :], AX, add)
                rsum = wk.tile([1, NL, 1], f32, tag="rsum")
                nc.vector.reciprocal_approx_fast(rsum[:, :, :], ssum[:, :, :])
                w_t = wk.tile([1, NL, M], bf16, tag="w")
                nc.vector.tensor_tensor(
                    w_t[:, :, :], r_t[:, :, :],
                    rsum[:, :, :].broadcast_to([1, NL, M]), mult)

                # -- w to partitions, block-diag stationary S_g
                s_g = wk.tile([128, NG, NL], bf16, tag="sg_w")
                for g in range(NG):
                    nc.tensor.transpose(wcol[g],
                                        w_t[0:1, g * 8:(g + 1) * 8, :],
                                        oc_s[0:1, 0:1])
                    nc.vector.tensor_tensor(
                        s_g[:, g, :], m0_s[:, g, :],
                        wcol[g].broadcast_to([128, NL]), mult)

                # -- attention contribution: psa += S_g^T . B_g, bank stops
                h_out = hop.tile([NL, H], bf16, tag="h")
                c_nxt = [None] * 4
                sgs = []
                for jt in range(4):
                    sl = slice(jt * 512, (jt + 1) * 512)
                    nc.tensor.matmul(psa_b[jt][p0:p0 + 16, :], s_g[:, 0, :],
                                     b_s[:, 0, sl], start=False, stop=False)
                    nc.tensor.matmul(psa_b[jt][p0:p0 + 16, :], s_g[:, 1, :],
                                     b_s[:, 1, sl], start=False, stop=True)
                    # bank jt complete: gates for H-slice jt
                    sg_t = wk.tile([NL, 384], bf16, tag=f"sg{jt}")
                    tg_t = wk.tile([NL, 128], bf16, tag=f"tg{jt}")
                    nc.scalar.activation(sg_t[:, :],
                                         psa_b[jt][p0:p0 + 16, 0:384],
                                         AF.Sigmoid)
                    nc.scalar.activation(tg_t[:, :],
                                         psa_b[jt][p0:p0 + 16, 384:512],
                                         AF.Tanh)
                    sgs.append((jt, sg_t, tg_t))
                    # c update for this slice
                    t1 = wk.tile([NL, 128], f32, tag=f"t1{jt}")
                    nc.gpsimd.tensor_tensor(t1[:, :], sg_t[:, 128:256],
                                            c_cur[jt][:, :], mult)
                    t2 = wk.tile([NL, 128], bf16, tag=f"t2{jt}")
                    nc.vector.tensor_tensor(t2[:, :], sg_t[:, 0:128],
                                            tg_t[:, :], mult)
                    c_nj = stp.tile([NL, 128], f32, tag=f"c{jt}", name="c_nj")
                    nc.vector.tensor_tensor(c_nj[:, :], t1[:, :],
                                            t2[:, :], add)
                    c_nxt[jt] = c_nj
                    # tanh(c) lagged by one bank for Act pipelining
                    if jt > 0:
                        pj, psg, _ = sgs[jt - 1]
                        pcs = slice(pj * 128, (pj + 1) * 128)
                        tc_t = wk.tile([NL, 128], bf16, tag=f"tc{pj}")
                        nc.scalar.activation(tc_t[:, :], c_nxt[pj][:, :],
                                             AF.Tanh)
                        nc.vector.tensor_tensor(h_out[:, pcs],
                                                psg[:, 256:384], tc_t[:, :],
                                                mult)
                pj, psg, _ = sgs[3]
                pcs = slice(pj * 128, (pj + 1) * 128)
                tc_t = wk.tile([NL, 128], bf16, tag=f"tc{pj}")
                nc.scalar.activation(tc_t[:, :], c_nxt[pj][:, :], AF.Tanh)
                nc.vector.tensor_tensor(h_out[:, pcs], psg[:, 256:384],
                                        tc_t[:, :], mult)

                nc.sync.dma_start(out=out_d.ap()[:, t, :], in_=h_out[:, :])

                if t < t_steps - 1:
                    # transpose chunk 0 of h_{t+1}; chunks 1-3 happen at the
                    # top of the next iteration
                    hT_next = stp.tile([128, KC, NL], bf16, tag="hT")
                    nc.tensor.transpose(pst_b[0][:, :], h_out[:, 0:128],
                                        id_s[:, :])
                    nc.vector.tensor_copy(hT_next[:, 0, :], pst_b[0][:, :])

                h_prev = h_out
                hT_cur = hT_next
                c_cur = c_nxt

    nc.compile()
    return nc


def _perm_cols(w):
    """Permute gate columns: new bank jt = [i_jt | f_jt | o_jt | g_jt]."""
    # w: (..., 4H) with original layout [i(512) | f | o | g]
    w4 = w.reshape(w.shape[:-1] + (4, 4, 128))   # (..., gate, jt, col)
    return np.ascontiguousarray(
        np.moveaxis(w4, -3, -2).reshape(w.shape))  # (..., jt, gate, col)


def _stage_inputs(x, A, Wx, Wh, Wattn, b, t_steps=T):
    """Shard + lay out inputs per core (host-side numpy staging)."""
    rb = NL * t_steps // 128
    h0 = A.mean(axis=(2, 3)).astype(np.float32)          # (N, H)
    ident = np.eye(NL, dtype=BF16)
    ones_col = np.ones((128, 1), dtype=BF16)
    m0 = np.zeros((128, NG, NL), dtype=BF16)
    for g in range(NG):
        for i in range(8):
            m0[i * 16:(i + 1) * 16, g, g * 8 + i] = 1

    def wlay(w):
        return np.ascontiguousarray(
            _perm_cols(w).astype(BF16).reshape(KC, 128, J).transpose(1, 0, 2))

    wxs, whs, was = wlay(Wx), wlay(Wh), wlay(Wattn)
    bvec = np.ascontiguousarray(_perm_cols(b.astype(np.float32))
                                .reshape(1, J))

    maps = []
    for k in range(NCORES):
        ns = slice(k * NL, (k + 1) * NL)
        x_sh = x[ns, :t_steps].astype(BF16)              # (NL, t, D)
        xT = x_sh.transpose(2, 0, 1).reshape(KC, 128, NL, rb, TB)
        xs_st = np.ascontiguousarray(
            xT.transpose(3, 1, 0, 4, 2).reshape(rb, 128, KC, 128))
        A_sh = A[ns].reshape(NL, H, M).astype(BF16)
        at_st = np.ascontiguousarray(
            A_sh.transpose(1, 0, 2).reshape(KC, 128, NL, M)
            .transpose(1, 0, 2, 3))
        h0_sh = h0[ns]                                    # (NL, H)
        h0t = np.ascontiguousarray(
            h0_sh.T.astype(BF16).reshape(KC, 128, NL).transpose(1, 0, 2))
        m = {
            "xs": xs_st, "at": at_st, "wx": wxs, "wh": whs, "wa": was,
            "h0t": h0t, "c0": np.ascontiguousarray(h0_sh),
            "ident": ident, "ones_col": ones_col, "m0": m0,
        }
        if np.any(b != 0):
            m["bvec"] = bvec
        maps.append(m)
    return maps


def _get_nc(has_bias, t_steps=T):
    key = (has_bias, t_steps)
    if key not in _CACHE:
        _CACHE[key] = build(t_steps=t_steps, has_bias=has_bias)
    return _CACHE[key]


def run_cores(x, A, Wx, Wh, Wattn, b, t_steps=T, trace=False):
    from concourse.bass_utils import run_bass_kernel_spmd
    maps = _stage_inputs(x, A, Wx, Wh, Wattn, b, t_steps=t_steps)
    has_bias = "bvec" in maps[0]
    nc = _get_nc(has_bias, t_steps)
    res = run_bass_kernel_spmd(nc, maps, list(range(NCORES)), trace=trace)
    out = np.concatenate(
        [np.asarray(res.results[k]["out"], dtype=np.float32)
         for k in range(NCORES)], axis=0)
    return out, res


def kernel(x, A, Wx, Wh, Wattn, b):
    x = np.asarray(x, dtype=np.float32)
    A = np.asarray(A, dtype=np.float32)
    out, _ = run_cores(x, A,
                       np.asarray(Wx, dtype=np.float32),
                       np.asarray(Wh, dtype=np.float32),
                       np.asarray(Wattn, dtype=np.float32),
                       np.asarray(b, dtype=np.float32))
    return out
